# revision 7
# baseline (speedup 1.0000x reference)
"""Causal MHA (RoPE, 16 heads, D=1024, S=2048, B=2) on 8 trn2 NeuronCores.

Sharding: data-parallel over batch (2 groups of 4 cores) x tensor-parallel
over heads (4 heads / core). Each core computes q/k/v projections for its
256 output dims, RoPE, causal attention for its 4 heads, and a partial
output projection y_c = out_c @ Wo[:, slice].T. Host sums the 4 partials
per batch (row-parallel unshard).

Fused software pipeline: QKV projections for token-quarter q+1 and the
deferred output projection are interleaved as PE filler inside attention
block q's key loop, so the tensor engine never idles (keeps the DVFS
p-state at max clock). All DMAs land directly in their compute tiles
(f32 tiles bitcast to float32r at matmul use; no staging casts).
"""

import math
from collections import deque

import numpy as np

D_MODEL = 1024
S = 2048
NH = 16
HD = 64
THETA = 10000.0
HPC = 4          # heads per core
DPC = HPC * HD   # dims per core = 256
NG = 2           # dim groups of 128 (pairs of heads)
W = 512          # q-block width
NKO = D_MODEL // 128
NTC = S // 128   # 16 token chunks of 128
MASK_VAL = -1e9
LAG = 2
PSUM_DMA = False  # DMA cannot read PSUM on trn2: stage y via SBUF

_CACHE = {}


def _build_nc():
    import concourse.bass as bass
    import concourse.tile as tile
    from concourse import bacc, mybir
    from contextlib import ExitStack

    F32 = mybir.dt.float32
    F32R = mybir.dt.float32r
    BF16 = mybir.dt.bfloat16
    AF = mybir.ActivationFunctionType
    MUL = mybir.AluOpType.mult
    ADD = mybir.AluOpType.add
    ts = bass.ts

    nc = bacc.Bacc(None, target_bir_lowering=False)
    xT = nc.dram_tensor("xT", [D_MODEL, S], F32R, kind="ExternalInput")
    wq = nc.dram_tensor("wq", [D_MODEL, DPC], F32R, kind="ExternalInput")
    wk = nc.dram_tensor("wk", [D_MODEL, DPC], F32R, kind="ExternalInput")
    wv = nc.dram_tensor("wv", [D_MODEL, DPC], F32R, kind="ExternalInput")
    wo = nc.dram_tensor("wo", [DPC, D_MODEL], F32R, kind="ExternalInput")
    coss = nc.dram_tensor("coss", [128, S], F32, kind="ExternalInput")
    sins = nc.dram_tensor("sins", [128, S], F32, kind="ExternalInput")
    pmat = nc.dram_tensor("pmat", [128, 128], F32R, kind="ExternalInput")
    mask = nc.dram_tensor("mask", [128, 2 * 128], F32, kind="ExternalInput")
    y = nc.dram_tensor("y", [S, D_MODEL], F32, kind="ExternalOutput")

    with tile.TileContext(nc) as tc, ExitStack() as ctx:
        const = ctx.enter_context(tc.tile_pool(name="const", bufs=1))
        persist = ctx.enter_context(tc.tile_pool(name="persist", bufs=1))
        work = ctx.enter_context(tc.tile_pool(name="work", bufs=2))
        ps_big = ctx.enter_context(
            tc.tile_pool(name="ps_big", bufs=2, space="PSUM"))
        ps_av = ctx.enter_context(
            tc.tile_pool(name="ps_av", bufs=1, space="PSUM"))

        # ---- persistent tiles ----------------------------------------
        wq_r = persist.tile([128, NKO, DPC], F32R, name="wq_r")
        wk_r = persist.tile([128, NKO, DPC], F32R, name="wk_r")
        wv_r = persist.tile([128, NKO, DPC], F32R, name="wv_r")
        wo_r = persist.tile([128, NG, D_MODEL], F32R, name="wo_r")
        cs_t = persist.tile([128, S], F32, name="cs_t")
        sn_t = persist.tile([128, S], F32, name="sn_t")
        pm_t = const.tile([128, 128], F32R, name="pm_t")
        msk01 = const.tile([128, 2, 128], F32, name="msk01")

        qT = [persist.tile([128, S], F32R, name=f"qT{g}") for g in range(NG)]
        kT = [persist.tile([128, S], F32R, name=f"kT{g}") for g in range(NG)]
        v_aug = persist.tile([128, NTC, HPC * (HD + 1)], F32R, name="v_aug")
        out_cT = [persist.tile([128, S], F32R, name=f"out_cT{g}")
                  for g in range(NG)]

        # ---- DMA prologue --------------------------------------------
        xT_v = xT.ap().rearrange("(c p) s -> p c s", p=128)  # [128, 8, S]

        xq_tiles = {}

        def emit_xq_dma(hf):
            t = work.tile([128, NKO, W], F32R, tag="xq", bufs=2, name="xq")
            for half in range(2):
                nc.sync.dma_start(
                    t[:, ts(half, 4)],
                    xT_v[:, ts(half, 4), ts(hf, W)])
            xq_tiles[hf] = t

        for name, dram, dst in (("wq", wq, wq_r), ("wk", wk, wk_r),
                                ("wv", wv, wv_r)):
            nc.scalar.dma_start(
                dst[:], dram.ap().rearrange("(c p) w -> p c w", p=128))
        emit_xq_dma(0)
        nc.scalar.dma_start(cs_t[:], coss.ap())
        nc.scalar.dma_start(sn_t[:], sins.ap())
        nc.scalar.dma_start(pm_t[:], pmat.ap())
        nc.scalar.dma_start(
            msk01[:].rearrange("p two c -> p (two c)"), mask.ap())
        nc.scalar.dma_start(wo_r[:], wo.ap().rearrange("(g p) e -> p g e",
                                                       p=128))
        emit_xq_dma(1)
        # ones columns of v_aug (denominator rows for the AV matmul)
        v4 = v_aug[:].rearrange("p t (h c) -> p t h c", h=HPC)
        ones64 = const.tile([128, NTC * HPC], F32, name="ones64")
        nc.gpsimd.memset(ones64[:], 1.0)
        nc.vector.tensor_copy(
            v4[:, :, :, HD:HD + 1],
            ones64[:].rearrange("p (t h one) -> p t h one", t=NTC, h=HPC))

        # ---- work units ----------------------------------------------
        def v_unit(hf, half):
            xq = xq_tiles[hf]
            psv = ps_big.tile([128, 2, DPC], F32, tag="big", name="psv")
            for j in range(2):
                tl = half * 2 + j
                for ko in range(NKO):
                    nc.tensor.matmul(psv[:, j], xq[:, ko, ts(tl, 128)],
                                     wv_r[:, ko],
                                     start=(ko == 0), stop=(ko == NKO - 1),
                                     skip_group_check=True)
            for j in range(2):
                tcN = hf * 4 + half * 2 + j
                nc.scalar.copy(
                    v4[:, tcN, :, 0:HD],
                    psv[:, j].rearrange("p (h c) -> p h c", h=HPC))

        rawqk_tiles = {}

        def qk_unit(hf, g):
            xq = xq_tiles[hf]
            psqk = ps_big.tile([128, 2 * W], F32, tag="big", name="psqk")
            for idx, wt in ((0, wq_r), (1, wk_r)):
                for ko in range(NKO):
                    nc.tensor.matmul(psqk[:, ts(idx, W)],
                                     wt[:, ko, ts(g, 128)],
                                     xq[:, ko],
                                     start=(ko == 0), stop=(ko == NKO - 1),
                                     skip_group_check=True)
            rawqk = work.tile([128, 2 * W], F32R, tag="rawqk", bufs=2,
                              name="rawqk")
            nc.scalar.copy(rawqk[:], psqk[:])
            rawqk_tiles[(hf, g)] = rawqk

        def rope_unit(hf, g):
            rawqk = rawqk_tiles.pop((hf, g))
            psp = ps_big.tile([128, 2 * W], F32, tag="big", name="psp")
            for idx in range(2):
                nc.tensor.matmul(psp[:, ts(idx, W)], pm_t[:],
                                 rawqk[:, ts(idx, W)],
                                 start=True, stop=True,
                                 skip_group_check=True)
            # t2 ops (the only PSUM readers) first, so psp's slot frees fast
            t2s = []
            for idx in range(2):
                t2 = work.tile([128, W], F32, tag="t2", bufs=2, name="t2")
                nc.vector.tensor_tensor(t2[:], psp[:, ts(idx, W)],
                                        sn_t[:, ts(hf, W)], MUL)
                t2s.append(t2)
            for idx, dst in ((0, qT[g]), (1, kT[g])):
                t1 = work.tile([128, W], F32, tag="t1", bufs=2, name="t1")
                nc.vector.tensor_tensor(t1[:], rawqk[:, ts(idx, W)],
                                        cs_t[:, ts(hf, W)], MUL)
                nc.vector.tensor_tensor(dst[:, ts(hf, W)], t1[:], t2s[idx][:],
                                        ADD)

        def qkv_units(hf):
            # V units for quarter hf are NOT here: they are emitted as
            # filler at the start of block hf (first consumer of its keys).
            units = []
            if hf + 1 < 4:
                units.append(lambda h=hf: emit_xq_dma(h + 1))
            for g in range(NG):
                units.append(lambda h=hf, g=g: qk_unit(h, g))
            for g in range(NG):
                units.append(lambda h=hf, g=g: rope_unit(h, g))
            return units

        def emit_oproj(tcN):
            psy = ps_big.tile([128, 2 * W], F32, tag="big", name="psy")
            for e2 in range(2):
                for g in range(NG):
                    nc.tensor.matmul(psy[:, ts(e2, W)],
                                     out_cT[g][:, ts(tcN, 128)],
                                     wo_r[:, g, ts(e2, W)],
                                     start=(g == 0), stop=(g == NG - 1),
                                     skip_group_check=True)
            if PSUM_DMA:
                nc.sync.dma_start(y.ap()[ts(tcN, 128), :], psy[:])
            else:
                ysb = work.tile([128, D_MODEL], F32, tag="ysb", bufs=2,
                                name="ysb")
                nc.vector.tensor_copy(ysb[:], psy[:])
                nc.sync.dma_start(y.ap()[ts(tcN, 128), :], ysb[:])

        # ---- fused attention pipeline --------------------------------
        # prologue: quarter 0 Q/K + RoPE (V(0) is block-0 filler)
        for g in range(NG):
            qk_unit(0, g)
        for g in range(NG):
            rope_unit(0, g)

        pending = []
        for qb in range(S // W):
            filler = deque()
            filler.append(lambda h=qb: v_unit(h, 0))
            filler.append(lambda h=qb: v_unit(h, 1))
            if qb + 1 < 4:
                filler.extend(qkv_units(qb + 1))

            nkb = (qb + 1) * (W // 128)
            av = [ps_av.tile([HD + 1, W], F32, tag=f"av{hh}",
                             name=f"av{hh}") for hh in range(4)]
            attq = deque()

            def emit_av(entry, nkb=nkb, av=av):
                kb, cs0, atts = entry
                for g in range(NG):
                    for h in range(2):
                        hh = 2 * g + h
                        nc.tensor.matmul(
                            av[hh][:, cs0:],
                            v_aug[:, kb, hh * (HD + 1):(hh + 1) * (HD + 1)],
                            atts[g][:, h * W + cs0:(h + 1) * W],
                            start=(kb == 0), stop=(kb == nkb - 1),
                            skip_group_check=True)

            for kb in range(nkb):
                if len(attq) >= LAG:
                    emit_av(attq.popleft())
                want = -(-len(filler) // (nkb - kb))  # ceil
                for _ in range(want):
                    if filler:
                        filler.popleft()()
                if pending:
                    emit_oproj(pending.pop(0))
                cs0 = max(0, kb * 128 - qb * W)
                diag = kb * 128 >= qb * W
                atts = []
                for g in range(NG):
                    sc = ps_big.tile([128, 2 * W], F32, tag="big", name="sc")
                    for h in range(2):
                        nc.tensor.matmul(
                            sc[:, h * W + cs0:(h + 1) * W],
                            kT[g][ts(h, HD), ts(kb, 128)],
                            qT[g][ts(h, HD), qb * W + cs0:(qb + 1) * W],
                            start=True, stop=True,
                            skip_group_check=True)
                    att = work.tile([128, 2 * W], F32R, tag="att", bufs=4,
                                    name="att")
                    scv = sc[:].rearrange("p (h w) -> p h w", h=2)
                    atv = att[:].rearrange("p (h w) -> p h w", h=2)
                    nc.scalar.activation(atv[:, :, cs0:], scv[:, :, cs0:],
                                         AF.Exp, scale=1.0 / math.sqrt(HD))
                    if diag:
                        band = atv[:, :, cs0:cs0 + 128]
                        nc.gpsimd.tensor_tensor(band, band, msk01[:], MUL)
                    atts.append(att)
                attq.append((kb, cs0, atts))
            while filler:
                filler.popleft()()
            while attq:
                emit_av(attq.popleft())

            # softmax normalize -> out_cT
            for hh in range(4):
                g, h = divmod(hh, 2)
                rs = work.tile([1, W], F32, tag="rs", bufs=2, name="rs")
                nc.vector.tensor_copy(rs[:], av[hh][HD:HD + 1, :])
                rec = work.tile([1, W], F32, tag="rec", bufs=2, name="rec")
                nc.vector.reciprocal_approx_fast(rec[:], rs[:])
                rb = work.tile([HD, W], F32, tag="rb", bufs=2, name="rb")
                nc.gpsimd.partition_broadcast(rb[:], rec[:])
                nc.vector.tensor_tensor(
                    out_cT[g][ts(h, HD), ts(qb, W)],
                    av[hh][0:HD, :], rb[:], MUL)
            pending.extend(qb * (W // 128) + tl for tl in range(W // 128))
        for tcN in pending:
            emit_oproj(tcN)

    nc.compile()
    return nc


def _host_inputs():
    d = HD
    inv_freq = THETA ** (-np.arange(0, d, 2, dtype=np.float64) / d)  # [32]
    t = np.arange(S, dtype=np.float64)
    ang = t[None, :] * inv_freq[:, None]          # [32, S]
    C64 = np.repeat(np.cos(ang), 2, axis=0)       # [64, S] per-dim cos
    S64 = np.repeat(np.sin(ang), 2, axis=0).copy()
    S64[0::2] *= -1.0                             # even dims: -sin
    C = np.tile(C64, (2, 1)).astype(np.float32)   # [128, S] two heads
    Sg = np.tile(S64, (2, 1)).astype(np.float32)

    P = np.zeros((128, 128), np.float32)
    idx = np.arange(128)
    P[idx ^ 1, idx] = 1.0

    M01 = np.where(np.arange(128)[None, :] >= np.arange(128)[:, None],
                   1.0, 0.0).astype(np.float32)
    M = np.tile(M01, (1, 2))  # [128, 256]: one copy per head of the pair
    return C, Sg, P, M


def kernel(x, Wq, Wk, Wv, Wo):
    from concourse.bass_utils import run_bass_kernel_spmd

    x = np.asarray(x, np.float32)
    Wq = np.asarray(Wq, np.float32)
    Wk = np.asarray(Wk, np.float32)
    Wv = np.asarray(Wv, np.float32)
    Wo = np.asarray(Wo, np.float32)
    B = x.shape[0]

    if "nc" not in _CACHE:
        _CACHE["nc"] = _build_nc()
    nc = _CACHE["nc"]

    C, Sg, P, M = _host_inputs()
    xTb = [np.ascontiguousarray(x[b].T) for b in range(B)]
    in_maps = []
    for c in range(8):
        b, hq = divmod(c, 4)
        sl = slice(hq * DPC, (hq + 1) * DPC)
        in_maps.append({
            "xT": xTb[b],
            "wq": np.ascontiguousarray(Wq[sl, :].T),
            "wk": np.ascontiguousarray(Wk[sl, :].T),
            "wv": np.ascontiguousarray(Wv[sl, :].T),
            "wo": np.ascontiguousarray(Wo[:, sl].T),
            "coss": C, "sins": Sg, "pmat": P, "mask": M,
        })

    res = run_bass_kernel_spmd(nc, in_maps, list(range(8)),
                               **_CACHE.get("runkw", {}))
    _CACHE["last_res"] = res
    out = np.zeros((B, S, D_MODEL), np.float32)
    for c in range(8):
        b = c // 4
        out[b] += res.results[c]["y"]
    return out


# revision 9
# speedup vs baseline: 1.1385x; 1.1385x over previous
"""Causal MHA (RoPE, 16 heads, D=1024, S=2048, B=2) on 8 trn2 NeuronCores.

Sharding: data-parallel over batch (2 groups of 4 cores) x tensor-parallel
over heads (4 heads / core). Each core computes q/k/v projections for its
256 output dims, RoPE, causal attention for its 4 heads, and a partial
output projection y_c = out_c @ Wo[:, slice].T. Host sums the 4 partials
per batch (row-parallel unshard).

Fused software pipeline: QKV projections for token-quarter q+1 and the
deferred output projection are interleaved as PE filler inside attention
block q's key loop, so the tensor engine never idles (keeps the DVFS
p-state at max clock). All DMAs land directly in their compute tiles
(f32 tiles bitcast to float32r at matmul use; no staging casts).
"""

import math
from collections import deque

import numpy as np

D_MODEL = 1024
S = 2048
NH = 16
HD = 64
THETA = 10000.0
HPC = 4          # heads per core
DPC = HPC * HD   # dims per core = 256
NG = 2           # dim groups of 128 (pairs of heads)
W = 512          # q-block width
NKO = D_MODEL // 128
NTC = S // 128   # 16 token chunks of 128
MASK_VAL = -1e9
LAG = 2
PSUM_DMA = False  # DMA cannot read PSUM on trn2: stage y via SBUF

_CACHE = {}


def _build_nc():
    import concourse.bass as bass
    import concourse.tile as tile
    from concourse import bacc, mybir
    from contextlib import ExitStack

    F32 = mybir.dt.float32
    F32R = mybir.dt.float32r
    BF16 = mybir.dt.bfloat16
    AF = mybir.ActivationFunctionType
    MUL = mybir.AluOpType.mult
    ADD = mybir.AluOpType.add
    ts = bass.ts

    nc = bacc.Bacc(None, target_bir_lowering=False)
    xT = nc.dram_tensor("xT", [D_MODEL, S], F32R, kind="ExternalInput")
    wq = nc.dram_tensor("wq", [D_MODEL, DPC], F32R, kind="ExternalInput")
    wk = nc.dram_tensor("wk", [D_MODEL, DPC], F32R, kind="ExternalInput")
    wv = nc.dram_tensor("wv", [D_MODEL, DPC], F32R, kind="ExternalInput")
    wo = nc.dram_tensor("wo", [DPC, D_MODEL], F32R, kind="ExternalInput")
    coss = nc.dram_tensor("coss", [128, S], F32, kind="ExternalInput")
    sins = nc.dram_tensor("sins", [128, S], F32, kind="ExternalInput")
    pmat = nc.dram_tensor("pmat", [128, 128], F32R, kind="ExternalInput")
    mask = nc.dram_tensor("mask", [128, 128], F32, kind="ExternalInput")
    ident = nc.dram_tensor("ident", [128, 128], F32, kind="ExternalInput")
    y = nc.dram_tensor("y", [S, D_MODEL], F32, kind="ExternalOutput")

    with tile.TileContext(nc) as tc, ExitStack() as ctx:
        const = ctx.enter_context(tc.tile_pool(name="const", bufs=1))
        persist = ctx.enter_context(tc.tile_pool(name="persist", bufs=1))
        work = ctx.enter_context(tc.tile_pool(name="work", bufs=2))
        ps_big = ctx.enter_context(
            tc.tile_pool(name="ps_big", bufs=2, space="PSUM"))
        ps_av = ctx.enter_context(
            tc.tile_pool(name="ps_av", bufs=1, space="PSUM"))

        # ---- persistent tiles ----------------------------------------
        wq_r = persist.tile([128, NKO, DPC], F32R, name="wq_r")
        wk_r = persist.tile([128, NKO, DPC], F32R, name="wk_r")
        wv_r = persist.tile([128, NKO, DPC], F32R, name="wv_r")
        wo_r = persist.tile([128, NG, D_MODEL], F32R, name="wo_r")
        cs_t = persist.tile([128, S], F32, name="cs_t")
        sn_t = persist.tile([128, S], F32, name="sn_t")
        pm_t = const.tile([128, 128], F32R, name="pm_t")
        msk_raw = const.tile([128, 128], F32, name="msk_raw")
        id_raw = const.tile([128, 128], F32, name="id_raw")
        msk_b = const.tile([128, 128], BF16, name="msk_b")
        id_b = const.tile([128, 128], BF16, name="id_b")

        qT = [persist.tile([128, S], F32R, name=f"qT{g}") for g in range(NG)]
        kT = [persist.tile([128, S], F32R, name=f"kT{g}") for g in range(NG)]
        v_aug = persist.tile([128, NTC, HPC * (HD + 1)], F32R, name="v_aug")
        out_cT = [persist.tile([128, S], F32R, name=f"out_cT{g}")
                  for g in range(NG)]

        # ---- DMA prologue --------------------------------------------
        xT_v = xT.ap().rearrange("(c p) s -> p c s", p=128)  # [128, 8, S]

        xq_tiles = {}

        def emit_xq_dma(hf):
            t = work.tile([128, NKO, W], F32R, tag="xq", bufs=2, name="xq")
            for half in range(2):
                nc.sync.dma_start(
                    t[:, ts(half, 4)],
                    xT_v[:, ts(half, 4), ts(hf, W)])
            xq_tiles[hf] = t

        for name, dram, dst in (("wq", wq, wq_r), ("wk", wk, wk_r),
                                ("wv", wv, wv_r)):
            nc.scalar.dma_start(
                dst[:], dram.ap().rearrange("(c p) w -> p c w", p=128))
        emit_xq_dma(0)
        nc.scalar.dma_start(cs_t[:], coss.ap())
        nc.scalar.dma_start(sn_t[:], sins.ap())
        nc.scalar.dma_start(pm_t[:], pmat.ap())
        nc.scalar.dma_start(msk_raw[:], mask.ap())
        nc.scalar.dma_start(id_raw[:], ident.ap())
        nc.scalar.dma_start(wo_r[:], wo.ap().rearrange("(g p) e -> p g e",
                                                       p=128))
        emit_xq_dma(1)
        nc.scalar.copy(msk_b[:], msk_raw[:])
        nc.scalar.copy(id_b[:], id_raw[:])
        # ones columns of v_aug (denominator rows for the AV matmul)
        v4 = v_aug[:].rearrange("p t (h c) -> p t h c", h=HPC)
        ones64 = const.tile([128, NTC * HPC], F32, name="ones64")
        nc.gpsimd.memset(ones64[:], 1.0)
        nc.vector.tensor_copy(
            v4[:, :, :, HD:HD + 1],
            ones64[:].rearrange("p (t h one) -> p t h one", t=NTC, h=HPC))

        # ---- work units ----------------------------------------------
        def v_unit(hf, half):
            xq = xq_tiles[hf]
            psv = ps_big.tile([128, 2, DPC], F32, tag="big", name="psv")
            for j in range(2):
                tl = half * 2 + j
                for ko in range(NKO):
                    nc.tensor.matmul(psv[:, j], xq[:, ko, ts(tl, 128)],
                                     wv_r[:, ko],
                                     start=(ko == 0), stop=(ko == NKO - 1),
                                     skip_group_check=True)
            for j in range(2):
                tcN = hf * 4 + half * 2 + j
                nc.scalar.copy(
                    v4[:, tcN, :, 0:HD],
                    psv[:, j].rearrange("p (h c) -> p h c", h=HPC))

        rawqk_tiles = {}

        def qk_unit(hf, g):
            xq = xq_tiles[hf]
            psqk = ps_big.tile([128, 2 * W], F32, tag="big", name="psqk")
            for idx, wt in ((0, wq_r), (1, wk_r)):
                for ko in range(NKO):
                    nc.tensor.matmul(psqk[:, ts(idx, W)],
                                     wt[:, ko, ts(g, 128)],
                                     xq[:, ko],
                                     start=(ko == 0), stop=(ko == NKO - 1),
                                     skip_group_check=True)
            rawqk = work.tile([128, 2 * W], F32R, tag="rawqk", bufs=2,
                              name="rawqk")
            nc.scalar.copy(rawqk[:], psqk[:])
            rawqk_tiles[(hf, g)] = rawqk

        def rope_unit(hf, g):
            rawqk = rawqk_tiles.pop((hf, g))
            psp = ps_big.tile([128, 2 * W], F32, tag="big", name="psp")
            for idx in range(2):
                nc.tensor.matmul(psp[:, ts(idx, W)], pm_t[:],
                                 rawqk[:, ts(idx, W)],
                                 start=True, stop=True,
                                 skip_group_check=True)
            # t2 ops (the only PSUM readers) first, so psp's slot frees fast
            t2s = []
            for idx in range(2):
                t2 = work.tile([128, W], F32, tag="t2", bufs=2, name="t2")
                nc.vector.tensor_tensor(t2[:], psp[:, ts(idx, W)],
                                        sn_t[:, ts(hf, W)], MUL)
                t2s.append(t2)
            for idx, dst in ((0, qT[g]), (1, kT[g])):
                t1 = work.tile([128, W], F32, tag="t1", bufs=2, name="t1")
                nc.vector.tensor_tensor(t1[:], rawqk[:, ts(idx, W)],
                                        cs_t[:, ts(hf, W)], MUL)
                nc.vector.tensor_tensor(dst[:, ts(hf, W)], t1[:], t2s[idx][:],
                                        ADD)

        def qkv_units(hf):
            # V units for quarter hf are NOT here: they are emitted as
            # filler at the start of block hf (first consumer of its keys).
            units = []
            if hf + 1 < 4:
                units.append(lambda h=hf: emit_xq_dma(h + 1))
            for g in range(NG):
                units.append(lambda h=hf, g=g: qk_unit(h, g))
            for g in range(NG):
                units.append(lambda h=hf, g=g: rope_unit(h, g))
            return units

        def emit_oproj(tcN):
            psy = ps_big.tile([128, 2 * W], F32, tag="big", name="psy")
            for e2 in range(2):
                for g in range(NG):
                    nc.tensor.matmul(psy[:, ts(e2, W)],
                                     out_cT[g][:, ts(tcN, 128)],
                                     wo_r[:, g, ts(e2, W)],
                                     start=(g == 0), stop=(g == NG - 1),
                                     skip_group_check=True)
            if PSUM_DMA:
                nc.sync.dma_start(y.ap()[ts(tcN, 128), :], psy[:])
            else:
                ysb = work.tile([128, D_MODEL], F32, tag="ysb", bufs=2,
                                name="ysb")
                nc.scalar.copy(ysb[:], psy[:])
                nc.sync.dma_start(y.ap()[ts(tcN, 128), :], ysb[:])

        # ---- fused attention pipeline --------------------------------
        # prologue: quarter 0 Q/K + RoPE (V(0) is block-0 filler)
        for g in range(NG):
            qk_unit(0, g)
        for g in range(NG):
            rope_unit(0, g)

        pending = []
        for qb in range(S // W):
            filler = deque()
            filler.append(lambda h=qb: v_unit(h, 0))
            filler.append(lambda h=qb: v_unit(h, 1))
            if qb + 1 < 4:
                filler.extend(qkv_units(qb + 1))

            nkb = (qb + 1) * (W // 128)
            av = [ps_av.tile([HD + 1, W], F32, tag=f"av{hh}",
                             name=f"av{hh}") for hh in range(4)]
            attq = deque()

            def emit_av(entry, nkb=nkb, av=av):
                kb, cs0, atts = entry
                for g in range(NG):
                    for h in range(2):
                        hh = 2 * g + h
                        nc.tensor.matmul(
                            av[hh][:, cs0:],
                            v_aug[:, kb, hh * (HD + 1):(hh + 1) * (HD + 1)],
                            atts[g][:, h * W + cs0:(h + 1) * W],
                            start=(kb == 0), stop=(kb == nkb - 1),
                            skip_group_check=True)

            for kb in range(nkb):
                if len(attq) >= LAG:
                    emit_av(attq.popleft())
                want = -(-len(filler) // (nkb - kb))  # ceil
                for _ in range(want):
                    if filler:
                        filler.popleft()()
                if pending:
                    emit_oproj(pending.pop(0))
                cs0 = max(0, kb * 128 - qb * W)
                diag = kb * 128 >= qb * W
                atts = []
                for g in range(NG):
                    sc = ps_big.tile([128, 2 * W], F32, tag="big", name="sc")
                    scv = sc[:].rearrange("p (h w) -> p h w", h=2)
                    for h in range(2):
                        nc.tensor.matmul(
                            sc[:, h * W + cs0:(h + 1) * W],
                            kT[g][ts(h, HD), ts(kb, 128)],
                            qT[g][ts(h, HD), qb * W + cs0:(qb + 1) * W],
                            start=True, stop=not diag,
                            skip_group_check=True)
                    if diag:
                        for h in range(2):
                            nc.tensor.matmul(
                                sc[:, h * W + cs0:h * W + cs0 + 128],
                                id_b[:], msk_b[:],
                                start=False, stop=True,
                                skip_group_check=True)
                    att = work.tile([128, 2 * W], F32R, tag="att", bufs=4,
                                    name="att")
                    atv = att[:].rearrange("p (h w) -> p h w", h=2)
                    nc.scalar.activation(atv[:, :, cs0:], scv[:, :, cs0:],
                                         AF.Exp, scale=1.0 / math.sqrt(HD))
                    atts.append(att)
                attq.append((kb, cs0, atts))
            while filler:
                filler.popleft()()
            while attq:
                emit_av(attq.popleft())

            # softmax normalize -> out_cT
            for hh in range(4):
                g, h = divmod(hh, 2)
                rs = work.tile([1, W], F32, tag="rs", bufs=2, name="rs")
                nc.vector.tensor_copy(rs[:], av[hh][HD:HD + 1, :])
                rec = work.tile([1, W], F32, tag="rec", bufs=2, name="rec")
                nc.vector.reciprocal_approx_fast(rec[:], rs[:])
                rb = work.tile([HD, W], F32, tag="rb", bufs=2, name="rb")
                nc.gpsimd.partition_broadcast(rb[:], rec[:])
                nc.vector.tensor_tensor(
                    out_cT[g][ts(h, HD), ts(qb, W)],
                    av[hh][0:HD, :], rb[:], MUL)
            pending.extend(qb * (W // 128) + tl for tl in range(W // 128))
        for tcN in pending:
            emit_oproj(tcN)

    nc.compile()
    return nc


def _host_inputs():
    d = HD
    inv_freq = THETA ** (-np.arange(0, d, 2, dtype=np.float64) / d)  # [32]
    t = np.arange(S, dtype=np.float64)
    ang = t[None, :] * inv_freq[:, None]          # [32, S]
    C64 = np.repeat(np.cos(ang), 2, axis=0)       # [64, S] per-dim cos
    S64 = np.repeat(np.sin(ang), 2, axis=0).copy()
    S64[0::2] *= -1.0                             # even dims: -sin
    C = np.tile(C64, (2, 1)).astype(np.float32)   # [128, S] two heads
    Sg = np.tile(S64, (2, 1)).astype(np.float32)

    P = np.zeros((128, 128), np.float32)
    idx = np.arange(128)
    P[idx ^ 1, idx] = 1.0

    M = np.where(np.arange(128)[None, :] >= np.arange(128)[:, None],
                 0.0, MASK_VAL).astype(np.float32)
    ident = np.eye(128, dtype=np.float32)
    return C, Sg, P, M, ident


def kernel(x, Wq, Wk, Wv, Wo):
    from concourse.bass_utils import run_bass_kernel_spmd

    x = np.asarray(x, np.float32)
    Wq = np.asarray(Wq, np.float32)
    Wk = np.asarray(Wk, np.float32)
    Wv = np.asarray(Wv, np.float32)
    Wo = np.asarray(Wo, np.float32)
    B = x.shape[0]

    if "nc" not in _CACHE:
        _CACHE["nc"] = _build_nc()
    nc = _CACHE["nc"]

    C, Sg, P, M, ident = _host_inputs()
    xTb = [np.ascontiguousarray(x[b].T) for b in range(B)]
    in_maps = []
    for c in range(8):
        b, hq = divmod(c, 4)
        sl = slice(hq * DPC, (hq + 1) * DPC)
        in_maps.append({
            "xT": xTb[b],
            "wq": np.ascontiguousarray(Wq[sl, :].T),
            "wk": np.ascontiguousarray(Wk[sl, :].T),
            "wv": np.ascontiguousarray(Wv[sl, :].T),
            "wo": np.ascontiguousarray(Wo[:, sl].T),
            "coss": C, "sins": Sg, "pmat": P, "mask": M, "ident": ident,
        })

    res = run_bass_kernel_spmd(nc, in_maps, list(range(8)),
                               **_CACHE.get("runkw", {}))
    _CACHE["last_res"] = res
    out = np.zeros((B, S, D_MODEL), np.float32)
    for c in range(8):
        b = c // 4
        out[b] += res.results[c]["y"]
    return out
